# revision 23
# baseline (speedup 1.0000x reference)
"""DeepseekV2 decoder layer on 8 TRN2 NeuronCores (Bass/Tile).

Sharding: TP over heads (2/core) for q/kv_b/attention/o_proj; kv_a + input
norm are token-sharded (each core computes its 256-token slice, then one
AllGather); TP over INTER (1024/core) for the MLP. 8 token chunks of 256;
attention and MLP emission fully interleaved; AllReduce/ReduceScatter run
on chunk PAIRS (512 tokens) to amortize collective overhead while staying
overlapped with compute.

Internal layout is feature-major ("transposed"): activations live as
[feature, token] so every matmul output feeds the next as `rhs` without any
on-device transpose. RoPE pair-swaps, RMSNorm weight folding, the softmax
scaling, and cos/sin tables are all folded into host-side weight prep.
"""

import numpy as np
import ml_dtypes

import concourse.bass as bass
import concourse.mybir as mybir
import concourse.tile as tile
from concourse import bacc
from concourse.bass_utils import run_bass_kernel_spmd

BF = ml_dtypes.bfloat16

B, S, HID = 2, 1024, 2048
T = B * S                      # 2048 tokens
H = 16
DN, DR = 128, 64
DQK = DN + DR
DV = 128
KVR = 512
INTER = 8192
EPS = 1e-6
ROPE_BASE = 10000.0
SCALING = DQK ** -0.5

NC_N = 8
HPC = H // NC_N                # 2 heads per core
FPC = INTER // NC_N            # 1024 inter per core
P = 128
HCH = HID // P                 # 16 hid chunks
HH = HCH // 2                  # 8 hid chunks per ht half
NCH = 8                        # token chunks
TW = T // NCH                  # 256 tokens per chunk
TW2 = 2 * TW                   # 512-token collective pairs
TWC = T // NC_N                # 256 tokens per core (kv_a shard)
QPB = S // TW                  # 4 q-chunks per batch
KVC = KVR // P                 # 4 kv_lora chunks
AGR = KVR + DR + 1             # 577 rows in the kv_a allgather payload
NEG = -30000.0

f32 = mybir.dt.float32
bf16 = mybir.dt.bfloat16
ADD = mybir.AluOpType.add
MUL = mybir.AluOpType.mult
BYP = mybir.AluOpType.bypass
AF = mybir.ActivationFunctionType

_CACHE = {}


def _build():
    nc = bacc.Bacc("TRN2", target_bir_lowering=False, debug=False, num_devices=NC_N)
    dp = lambda n, sh, dt: nc.dram_tensor(n, sh, dt, kind="ExternalInput")
    htb = dp("htb", [HID, T], bf16)
    htc = dp("htc", [HID, TWC], bf16)           # this core's token slice
    wq = dp("wq", [HID, HPC * DQK], bf16)       # [h0n,h1n,h0x1,h0x2,h1x1,h1x2]
    wkva = dp("wkva", [HID, KVR + DR], bf16)
    wkvbn = dp("wkvbn", [KVR, H * DN], bf16)
    wkvbv = dp("wkvbv", [KVR, H * DV], bf16)
    wo = dp("wo", [HPC * DV, HID], bf16)
    wg = dp("wg", [HID, FPC], bf16)
    wu = dp("wu", [HID, FPC], bf16)
    wd = dp("wd", [FPC, HID], bf16)
    cosf = dp("cosf", [DR, T], bf16)
    sinf = dp("sinf", [DR, T], bf16)
    csc = dp("csc", [DR, TWC], bf16)
    snc = dp("snc", [DR, TWC], bf16)
    masks = dp("masks", [P, 2, TW], f32)
    mo = nc.dram_tensor("mo", [HID // NC_N, T], bf16, kind="ExternalOutput")
    rg = [list(range(NC_N))]

    r2 = lambda ap: ap.rearrange("(o p) t -> p o t", p=P)
    cols = lambda c: slice(c * TW, (c + 1) * TW)

    with tile.TileContext(nc) as tc:
        with tc.tile_pool(name="const", bufs=1) as cpool, \
             tc.tile_pool(name="dram", bufs=1, space="DRAM") as dram, \
             tc.tile_pool(name="wrk", bufs=2) as wrk, \
             tc.tile_pool(name="ps", bufs=1, space="PSUM") as ps:
            ones_col = cpool.tile([P, 1], bf16)
            nc.vector.memset(ones_col[:], 1.0)
            ones_row = cpool.tile([1, P], bf16)
            nc.vector.memset(ones_row[:], 1.0)
            epsb = cpool.tile([1, 1], f32)
            nc.vector.memset(epsb[:], EPS)

            ag_in = dram.tile([NC_N * KVR, TWC], bf16, name="ag_in")
            ag_out = dram.tile([NC_N * KVR, TWC], bf16, name="ag_out")
            ag2_in = dram.tile([DR + 1, TWC], bf16, name="ag2_in")
            ag2_out = dram.tile([NC_N * (DR + 1), TWC], bf16,
                                addr_space="Shared", name="ag2_out")
            NP = NCH // 2
            ar_in = [dram.tile([HID, TW2], bf16, name=f"ar_in{p}") for p in range(NP)]
            ar_out = [dram.tile([HID, TW2], bf16, addr_space="Shared",
                                name=f"ar_out{p}") for p in range(NP)]
            rs_in = [dram.tile([HID, TW2], bf16, name=f"rs_in{p}") for p in range(NP)]
            rs_out = [dram.tile([HID // NC_N, TW2], bf16, name=f"rs_out{p}")
                      for p in range(NP)]
            rsl_in = [dram.tile([HID, TW], bf16, name=f"rsl_in{j}")
                      for j in range(2)]
            rsl_out = [dram.tile([HID // NC_N, TW], bf16, name=f"rsl_out{j}")
                      for j in range(2)]

            with tc.tile_pool(name="mlpw", bufs=1) as mlpw, \
                 tc.tile_pool(name="keepA", bufs=1) as keepA:
                wg_sb = mlpw.tile([P, HCH, FPC], bf16)       # 32K
                wu_sb = mlpw.tile([P, HCH, FPC], bf16)       # 32K
                wd_sb = mlpw.tile([P, FPC // P, HID], bf16)  # 32K

                qsb = keepA.tile([P, HPC, T], bf16)          # 8K
                qr = [keepA.tile([DR, T], bf16, name=f"qr{h}") for h in range(HPC)]
                krope = keepA.tile([DR, T], bf16)
                knope = keepA.tile([P, HPC, T], bf16)        # 8K
                vnat = keepA.tile([P, TWC // P, NC_N, HPC * DV], bf16)  # 8K
                wo_sb = keepA.tile([P, HPC, HID], bf16)      # 8K
                msk = keepA.tile([P, 2, TW], f32)            # 2K
                nc.sync.dma_start(wo_sb[:], r2(wo.ap()))
                nc.sync.dma_start(msk[:], masks.ap())

                # ==== A0: this core's kv_a/r1 shard + AllGather ====
                with tc.tile_pool(name="a1", bufs=1) as a1:
                    cs = a1.tile([DR, T], bf16)
                    sn = a1.tile([DR, T], bf16)
                    qsb2 = a1.tile([P, T], bf16)             # rope-input plane
                    r1row = a1.tile([1, T], bf16)

                    a1l_ctx = tc.tile_pool(name="a1l", bufs=1)
                    a1l = a1l_ctx.__enter__()
                    a1la_ctx = tc.tile_pool(name="a1la", bufs=1)
                    a1la = a1la_ctx.__enter__()
                    wkva_sb = a1la.tile([P, HCH, KVR + DR], bf16)  # 18K
                    htl = a1la.tile([P, HCH, TWC], bf16)           # 8K
                    csl = a1la.tile([DR, TWC], bf16)
                    snl = a1la.tile([DR, TWC], bf16)
                    nc.sync.dma_start(htl[:], r2(htc.ap()))
                    nc.sync.dma_start(wkva_sb[:], r2(wkva.ap()))
                    nc.sync.dma_start(csl[:], csc.ap())
                    nc.sync.dma_start(snl[:], snc.ap())
                    nc.sync.dma_start(cs[:], cosf.ap())
                    nc.sync.dma_start(sn[:], sinf.ap())

                    # local input-norm scale r1 for this core's tokens
                    ssl = ps.tile([1, TWC], f32, tag="row", bufs=2, name="ssl")
                    for o in range(HCH):
                        sq = wrk.tile([P, TWC], bf16, tag="sq", bufs=3, name="sql")
                        nc.scalar.square(sq[:], htl[:, o, :])
                        nc.tensor.matmul(ssl[:], ones_col[:], sq[:],
                                         start=(o == 0), stop=(o == HCH - 1))
                    srl = wrk.tile([1, TWC], f32, tag="frow", bufs=3, name="srl")
                    nc.scalar.activation(srl[:], ssl[:], AF.Sqrt,
                                         bias=epsb[:], scale=1.0 / HID)
                    rrl = wrk.tile([1, TWC], f32, tag="frow", bufs=3, name="rrl")
                    nc.vector.reciprocal_approx_fast(out=rrl[:], in_=srl[:])
                    rl = wrk.tile([1, TWC], bf16, tag="rb2", bufs=2, name="rl")
                    nc.vector.tensor_copy(out=rl[:], in_=rrl[:])
                    bclp = ps.tile([P, TWC], f32, tag="gu", bufs=2, name="bclp")
                    nc.tensor.matmul(bclp[:], ones_row[:], rl[:],
                                     start=True, stop=True)
                    bcl = wrk.tile([P, TWC], bf16, tag="bc", bufs=2, name="bcl")
                    nc.vector.tensor_copy(out=bcl[:], in_=bclp[:])

                    # local latent + 2nd rmsnorm + k_pe rope
                    kval = a1l.tile([P, KVC, TWC], bf16, name="kval")
                    kpel = a1l.tile([DR, TWC], bf16, name="kpel")
                    ss2 = ps.tile([1, TWC], f32, tag="row", bufs=2, name="ss2l")
                    for f in range(KVC + 1):
                        wid = P if f < KVC else DR
                        lp = ps.tile([P, TWC], f32, tag="big", bufs=2, name="lpl")
                        for o in range(HCH):
                            nc.tensor.matmul(lp[:wid, :],
                                             wkva_sb[:, o, f * P:f * P + wid],
                                             htl[:, o, :],
                                             start=(o == 0), stop=(o == HCH - 1))
                        if f < KVC:
                            sq2 = wrk.tile([P, TWC], bf16, tag="sq", bufs=3,
                                           name="sq2l")
                            nc.scalar.square(sq2[:], lp[:])
                            nc.tensor.matmul(ss2[:], ones_col[:], sq2[:],
                                             start=(f == 0), stop=(f == KVC - 1))
                            nc.vector.tensor_copy(out=kval[:, f, :], in_=lp[:])
                        else:
                            nc.vector.tensor_tensor(kpel[:], lp[:DR, :],
                                                    bcl[:DR, :], MUL)
                    sr2 = wrk.tile([1, TWC], f32, tag="frow", bufs=3, name="sr2l")
                    nc.scalar.activation(sr2[:], ss2[:], AF.Sqrt,
                                         bias=epsb[:], scale=1.0 / KVR)
                    rr2 = wrk.tile([1, TWC], f32, tag="frow", bufs=3, name="rr2l")
                    nc.vector.reciprocal_approx_fast(out=rr2[:], in_=sr2[:])
                    rb2 = wrk.tile([1, TWC], bf16, tag="rb2", bufs=2, name="rb2l")
                    nc.vector.tensor_copy(out=rb2[:], in_=rr2[:])
                    bcp2 = ps.tile([P, TWC], f32, tag="gu", bufs=2, name="bcp2l")
                    nc.tensor.matmul(bcp2[:], ones_row[:], rb2[:],
                                     start=True, stop=True)
                    bc2 = wrk.tile([P, TWC], bf16, tag="bc", bufs=2, name="bc2l")
                    nc.vector.tensor_copy(out=bc2[:], in_=bcp2[:])
                    for f in range(KVC):
                        nc.vector.tensor_tensor(kval[:, f, :], kval[:, f, :],
                                                bc2[:], MUL)

                    ksl = a1la.tile([DR, TWC], bf16, name="ksl")
                    nc.sync.dma_start(ksl[0:32, :], kpel[32:64, :])
                    nc.sync.dma_start(ksl[32:64, :], kpel[0:32, :])
                    ktl = a1la.tile([DR, TWC], bf16, name="ktl")
                    krl = a1l.tile([DR, TWC], bf16, name="krl")
                    nc.vector.tensor_tensor(ktl[:], kpel[:], csl[:], MUL)
                    nc.vector.tensor_tensor(krl[:], ksl[:], snl[:], MUL)
                    nc.vector.tensor_tensor(krl[:], krl[:], ktl[:], ADD)
                    a1la_ctx.__exit__(None, None, None)

                    # kv_b for this core's tokens x every dest core's heads
                    a1lb_ctx = tc.tile_pool(name="a1lb", bufs=1)
                    a1lb = a1lb_ctx.__enter__()
                    wkvbn_sb = a1lb.tile([P, KVC, H * DN], bf16, name="wkvbn_sb")
                    wkvbv_sb = a1lb.tile([P, KVC, H * DV], bf16, name="wkvbv_sb")
                    nc.sync.dma_start(wkvbn_sb[:], r2(wkvbn.ap()))
                    nc.sync.dma_start(wkvbv_sb[:], r2(wkvbv.ap()))
                    for dd in range(NC_N):
                        hc0 = dd * HPC * DN
                        klo = a1lb.tile([P, HPC, TWC], bf16, tag="klo", bufs=1,
                                        name="klo")
                        for h in range(HPC):
                            kp = ps.tile([P, TWC], f32, tag="big", bufs=2,
                                         name="kpl")
                            for f in range(KVC):
                                nc.tensor.matmul(
                                    kp[:],
                                    wkvbn_sb[:, f, hc0 + h * P:hc0 + (h + 1) * P],
                                    kval[:, f, :],
                                    start=(f == 0), stop=(f == KVC - 1))
                            nc.vector.tensor_copy(out=klo[:, h, :], in_=kp[:])
                        nc.sync.dma_start(
                            ag_in[dd * KVR:dd * KVR + HPC * P, :].rearrange(
                                "(h p) t -> p h t", p=P), klo[:])
                        vlo = a1lb.tile([P, TWC // P, HPC * DV], bf16, tag="vlo",
                                        bufs=1, name="vlo")
                        for tsub in range(TWC // P):
                            vp = ps.tile([P, HPC * DV], f32, tag="att", bufs=1,
                                         name="vpl")
                            for f in range(KVC):
                                nc.tensor.matmul(
                                    vp[:],
                                    kval[:, f, tsub * P:(tsub + 1) * P],
                                    wkvbv_sb[:, f, hc0:hc0 + HPC * DV],
                                    start=(f == 0), stop=(f == KVC - 1))
                            nc.vector.tensor_copy(out=vlo[:, tsub, :], in_=vp[:])
                        nc.sync.dma_start(
                            ag_in[dd * KVR + HPC * P:(dd + 1) * KVR, :].rearrange(
                                "(s p) t -> p s t", p=P), vlo[:])
                    nc.gpsimd.collective_compute(
                        "AllToAll", BYP, ins=[ag_in[:].opt()],
                        outs=[ag_out[:].opt()], replica_groups=rg)
                    a1lb_ctx.__exit__(None, None, None)


                    # allgather krope/r1 rows
                    nc.sync.dma_start(ag2_in[0:DR, :], krl[:])
                    nc.sync.dma_start(ag2_in[DR:DR + 1, :], rl[:])
                    nc.gpsimd.collective_compute(
                        "AllGather", BYP, ins=[ag2_in[:].opt()],
                        outs=[ag2_out[:].opt()], replica_groups=rg)
                    a1l_ctx.__exit__(None, None, None)

                    # ==== A1: q projection over all tokens ====
                    a1p_ctx = tc.tile_pool(name="a1p", bufs=1)
                    a1p = a1p_ctx.__enter__()
                    wq_sb = a1p.tile([P, HCH, HPC * DQK], bf16)   # 12K
                    nc.sync.dma_start(wq_sb[:], r2(wq.ap()))
                    for c in range(NCH):
                        tc0 = cols(c)
                        hts = []
                        for half in range(2):
                            hth = a1p.tile([P, HH, TW], bf16, tag="ht", bufs=3,
                                           name="ht")
                            nc.sync.dma_start(
                                hth[:],
                                r2(htb.ap()[half * P * HH:(half + 1) * P * HH, tc0]))
                            hts.append(hth)
                        ht = lambda o: hts[o // HH][:, o % HH, :]
                        for f in range(3):
                            qp = ps.tile([P, TW], f32, tag="big", bufs=2, name="qp")
                            for o in range(HCH):
                                nc.tensor.matmul(qp[:],
                                                 wq_sb[:, o, f * P:(f + 1) * P],
                                                 ht(o),
                                                 start=(o == 0), stop=(o == HCH - 1))
                            dst = qsb[:, f, tc0] if f < HPC else qsb2[:, tc0]
                            nc.vector.tensor_copy(out=dst, in_=qp[:])

                    # r1/krope rows from the second allgather
                    agv = ag2_out[:, :].rearrange("(c r) t -> r c t", c=NC_N)
                    nc.sync.dma_start(
                        r1row[:].rearrange("p (c t) -> p c t", c=NC_N),
                        agv[DR:DR + 1])
                    nc.sync.dma_start(
                        krope[:].rearrange("p (c t) -> p c t", c=NC_N),
                        agv[0:DR])
                    a1p_ctx.__exit__(None, None, None)

                    # unpack the alltoall: all tokens, this core's heads
                    agv1 = ag_out[:, :].rearrange("(c r) t -> r c t", c=NC_N)
                    for h in range(HPC):
                        nc.sync.dma_start(
                            knope[:, h, :].rearrange("p (c t) -> p c t", c=NC_N),
                            agv1[h * P:(h + 1) * P])
                    for s in range(TWC // P):
                        nc.sync.dma_start(
                            vnat[:, s, :, :],
                            agv1[HPC * P + s * P:HPC * P + (s + 1) * P])

                    # scale q in place by r1
                    for c in range(NCH):
                        tc0 = cols(c)
                        bcp = ps.tile([P, TW], f32, tag="gu", bufs=2, name="bcq")
                        nc.tensor.matmul(bcp[:], ones_row[:], r1row[:, tc0],
                                         start=True, stop=True)
                        bc1 = wrk.tile([P, TW], bf16, tag="bc", bufs=2, name="bc1")
                        nc.vector.tensor_copy(out=bc1[:], in_=bcp[:])
                        for f in range(HPC):
                            nc.vector.tensor_tensor(qsb[:, f, tc0], qsb[:, f, tc0],
                                                    bc1[:], MUL)
                        nc.vector.tensor_tensor(qsb2[:, tc0], qsb2[:, tc0],
                                                bc1[:], MUL)

                    # q rope: [x1(32); x2(32)] per head; pair-swap via sbuf dma
                    rope_ctx = tc.tile_pool(name="rope", bufs=4)
                    rp = rope_ctx.__enter__()
                    for h in range(HPC):
                        src = qsb2[:, :]
                        if h == 0:
                            direct = src[0:DR, :]
                        else:
                            dcp = rp.tile([DR, T], bf16, tag="rt", name="dcp")
                            nc.sync.dma_start(dcp[:], src[DR:2 * DR, :])
                            direct = dcp[:]
                        sw = rp.tile([DR, T], bf16, tag="rt", name=f"qsw{h}")
                        nc.sync.dma_start(sw[0:32, :], src[h * DR + 32:h * DR + 64, :])
                        nc.sync.dma_start(sw[32:64, :], src[h * DR:h * DR + 32, :])
                        tmp = rp.tile([DR, T], bf16, tag="rt", name=f"qtmp{h}")
                        nc.vector.tensor_tensor(tmp[:], direct, cs[:], MUL)
                        nc.vector.tensor_tensor(qr[h][:], sw[:], sn[:], MUL)
                        nc.vector.tensor_tensor(qr[h][:], qr[h][:], tmp[:], ADD)
                    rope_ctx.__exit__(None, None, None)

                # ============ fused attention + MLP ============
                with tc.tile_pool(name="aliv", bufs=1) as aliv, \
                     tc.tile_pool(name="mliv", bufs=1) as mliv:

                    def attn_chunk(c):
                        b, qt = c // QPB, c % QPB
                        qc0 = cols(c)
                        nkt = 2 * qt + 2
                        attn = aliv.tile([P, HPC, TW], bf16, tag="at", bufs=2,
                                         name="attn")
                        for h in range(HPC):
                            dnp = ps.tile([1, TW], f32, tag="row", bufs=2,
                                          name="dnp")
                            atp = ps.tile([P, TW], f32, tag="att", bufs=1,
                                          name="atp")
                            exs = [None] * nkt

                            def consume(kt):
                                nc.tensor.matmul(dnp[:], ones_col[:], exs[kt][:],
                                                 start=(kt == 0),
                                                 stop=(kt == nkt - 1))
                                to = b * (S // P) + kt
                                nc.tensor.matmul(atp[:],
                                                 vnat[:, to % 2, to // 2,
                                                      h * DV:(h + 1) * DV],
                                                 exs[kt][:],
                                                 start=(kt == 0),
                                                 stop=(kt == nkt - 1))

                            for kt in range(nkt):
                                kc0 = b * S + kt * P
                                scp = ps.tile([P, TW], f32, tag="big", bufs=2,
                                              name="scp")
                                nc.tensor.matmul(scp[:],
                                                 knope[:, h, kc0:kc0 + P],
                                                 qsb[:, h, qc0],
                                                 start=True, stop=False)
                                nc.tensor.matmul(scp[:],
                                                 krope[:, kc0:kc0 + P],
                                                 qr[h][:, qc0],
                                                 start=False, stop=True)
                                ex = wrk.tile([P, TW], bf16, tag="ex", bufs=4,
                                              name="ex")
                                j = kt - 2 * qt
                                if j >= 0:
                                    mtmp = wrk.tile([P, TW], f32, tag="mt",
                                                    bufs=2, name="mtmp")
                                    nc.vector.tensor_tensor(mtmp[:], scp[:],
                                                            msk[:, j, :], ADD)
                                    nc.scalar.activation(ex[:], mtmp[:], AF.Exp)
                                else:
                                    nc.scalar.activation(ex[:], scp[:], AF.Exp)
                                exs[kt] = ex
                                if kt >= 2:
                                    consume(kt - 2)
                            consume(max(nkt - 2, 0))
                            if nkt > 1:
                                consume(nkt - 1)
                            drow = wrk.tile([1, TW], f32, tag="frow", bufs=3,
                                            name="drow")
                            nc.vector.reciprocal_approx_fast(out=drow[:],
                                                             in_=dnp[:])
                            rbd = wrk.tile([1, TW], bf16, tag="rb2", bufs=2,
                                           name="rbd")
                            nc.vector.tensor_copy(out=rbd[:], in_=drow[:])
                            dbp = ps.tile([P, TW], f32, tag="gu", bufs=2,
                                          name="dbp")
                            nc.tensor.matmul(dbp[:], ones_row[:], rbd[:],
                                             start=True, stop=True)
                            dbc = wrk.tile([P, TW], f32, tag="mt", bufs=2,
                                           name="dbc")
                            nc.vector.tensor_copy(out=dbc[:], in_=dbp[:])
                            nc.vector.tensor_tensor(attn[:, h, :], atp[:],
                                                    dbc[:], MUL)
                        # o_proj partials into this pair's AR staging half
                        stg = mliv.tile([P, HCH, TW], bf16, tag="stg", bufs=1,
                                        name="ostg")
                        for ho in range(HCH):
                            op = ps.tile([P, TW], f32, tag="big", bufs=2,
                                         name="op")
                            for h in range(HPC):
                                nc.tensor.matmul(op[:],
                                                 wo_sb[:, h, ho * P:(ho + 1) * P],
                                                 attn[:, h, :],
                                                 start=(h == 0),
                                                 stop=(h == HPC - 1))
                            nc.vector.tensor_copy(out=stg[:, ho, :], in_=op[:])
                        half = (c % 2) * TW
                        nc.sync.dma_start(
                            r2(ar_in[c // 2][:, half:half + TW]), stg[:])

                    def ar_pair(p):
                        nc.gpsimd.collective_compute(
                            "AllReduce", ADD, ins=[ar_in[p][:].opt()],
                            outs=[ar_out[p][:].opt()], replica_groups=rg)

                    h2s, xs = {}, {}

                    def mlp_pre(c):
                        tc0 = cols(c)
                        half = (c % 2) * TW
                        # x = hidden + attn_out (in place over the AR dma)
                        x = mliv.tile([P, HCH, TW], bf16, tag="x", bufs=2,
                                      name="x")
                        nc.sync.dma_start(x[:],
                                          r2(ar_out[c // 2][:, half:half + TW]))
                        htre = mliv.tile([P, HCH, TW], bf16, tag="htre", bufs=1,
                                         name="htre")
                        nc.sync.dma_start(htre[:], r2(htb.ap()[:, tc0]))
                        nc.vector.tensor_tensor(x[:, 0:HH, :], x[:, 0:HH, :],
                                                htre[:, 0:HH, :], ADD)
                        nc.vector.tensor_tensor(x[:, HH:HCH, :], x[:, HH:HCH, :],
                                                htre[:, HH:HCH, :], ADD)
                        # post-attn rmsnorm scale
                        ssp3 = ps.tile([1, TW], f32, tag="row", bufs=2,
                                       name="ssp3")
                        for o in range(HCH):
                            sq3 = wrk.tile([P, TW], bf16, tag="sq", bufs=3,
                                           name="sq3")
                            nc.scalar.square(sq3[:], x[:, o, :])
                            nc.tensor.matmul(ssp3[:], ones_col[:], sq3[:],
                                             start=(o == 0), stop=(o == HCH - 1))
                        srow3 = wrk.tile([1, TW], f32, tag="frow", bufs=3,
                                         name="srow3")
                        nc.scalar.activation(srow3[:], ssp3[:], AF.Sqrt,
                                             bias=epsb[:], scale=1.0 / HID)
                        rrow3 = wrk.tile([1, TW], f32, tag="frow", bufs=3,
                                         name="rrow3")
                        nc.vector.reciprocal_approx_fast(out=rrow3[:],
                                                         in_=srow3[:])
                        rb3 = wrk.tile([1, TW], bf16, tag="rb2", bufs=2,
                                       name="rb3")
                        nc.vector.tensor_copy(out=rb3[:], in_=rrow3[:])
                        bcp3 = ps.tile([P, TW], f32, tag="gu", bufs=2,
                                       name="bcp3")
                        nc.tensor.matmul(bcp3[:], ones_row[:], rb3[:],
                                         start=True, stop=True)
                        bc3 = wrk.tile([P, TW], bf16, tag="bc", bufs=2,
                                       name="bc3")
                        nc.vector.tensor_copy(out=bc3[:], in_=bcp3[:])
                        h2 = mliv.tile([P, HCH, TW], bf16, tag="h2", bufs=1,
                                       name="h2")
                        for o in range(HCH):
                            nc.vector.tensor_tensor(h2[:, o, :], x[:, o, :],
                                                    bc3[:], MUL)
                        # x -> x/8 in place (folded residual for ReduceScatter)
                        nc.vector.tensor_scalar_mul(x[:], x[:], 0.125)
                        h2s[c], xs[c] = h2, x

                    def mlp_main(c):
                        h2, x = h2s.pop(c), xs.pop(c)
                        half = (c % 2) * TW
                        act = mliv.tile([P, FPC // P, TW], bf16, tag="act",
                                        bufs=1, name="act")
                        for fi in range(FPC // P):
                            gp = ps.tile([P, TW], f32, tag="gu", bufs=2,
                                         name="gp")
                            for o in range(HCH):
                                nc.tensor.matmul(gp[:],
                                                 wg_sb[:, o, fi * P:(fi + 1) * P],
                                                 h2[:, o, :],
                                                 start=(o == 0),
                                                 stop=(o == HCH - 1))
                            up = ps.tile([P, TW], f32, tag="gu", bufs=2,
                                         name="up")
                            for o in range(HCH):
                                nc.tensor.matmul(up[:],
                                                 wu_sb[:, o, fi * P:(fi + 1) * P],
                                                 h2[:, o, :],
                                                 start=(o == 0),
                                                 stop=(o == HCH - 1))
                            gs = wrk.tile([P, TW], bf16, tag="gs", bufs=2,
                                          name="gs")
                            nc.scalar.activation(gs[:], gp[:], AF.Silu)
                            nc.vector.tensor_tensor(act[:, fi, :], up[:],
                                                    gs[:], MUL)

                        # down projection partial (+x/8) into RS staging half
                        stg = mliv.tile([P, HCH, TW], bf16, tag="stg", bufs=1,
                                        name="dstg")
                        for ho in range(HCH):
                            dpp = ps.tile([P, TW], f32, tag="big", bufs=2,
                                          name="dpp")
                            for f in range(FPC // P):
                                nc.tensor.matmul(dpp[:],
                                                 wd_sb[:, f, ho * P:(ho + 1) * P],
                                                 act[:, f, :],
                                                 start=(f == 0),
                                                 stop=(f == FPC // P - 1))
                            nc.vector.tensor_tensor(stg[:, ho, :], dpp[:],
                                                    x[:, ho, :], ADD)
                        if c >= 6:
                            nc.sync.dma_start(r2(rsl_in[c - 6][:, :]), stg[:])
                        else:
                            nc.sync.dma_start(
                                r2(rs_in[c // 2][:, half:half + TW]), stg[:])

                    def rs_pair(p):
                        nc.gpsimd.collective_compute(
                            "ReduceScatter", ADD, ins=[rs_in[p][:].opt()],
                            outs=[rs_out[p][:].opt()], replica_groups=rg)
                        nc.gpsimd.dma_start(
                            mo.ap()[:, p * TW2:(p + 1) * TW2], rs_out[p][:, :])

                    def rs_last(j):
                        c = 6 + j
                        nc.gpsimd.collective_compute(
                            "ReduceScatter", ADD,
                            ins=[rsl_in[j][:].opt()],
                            outs=[rsl_out[j][:].opt()], replica_groups=rg)
                        nc.gpsimd.dma_start(
                            mo.ap()[:, cols(c)], rsl_out[j][:, :])

                    nc.sync.dma_start(wg_sb[:], r2(wg.ap()))
                    nc.sync.dma_start(wu_sb[:], r2(wu.ap()))
                    nc.sync.dma_start(wd_sb[:], r2(wd.ap()))

                    sched = [("A", 0), ("A", 1), ("R", 0), ("A", 2), ("A", 3),
                             ("R", 1), ("P", 0), ("M", 0), ("A", 4), ("P", 1),
                             ("M", 1), ("A", 5), ("R", 2), ("P", 2),
                             ("M", 2), ("A", 6), ("P", 3), ("M", 3),
                             ("A", 7), ("R", 3), ("P", 4), ("M", 4), ("P", 5),
                             ("M", 5), ("P", 6), ("M", 6), ("S", 0), ("S", 1),
                             ("S", 2), ("L", 0), ("P", 7), ("M", 7), ("L", 1)]
                    for kind, c in sched:
                        if kind == "A":
                            attn_chunk(c)
                        elif kind == "R":
                            ar_pair(c)
                        elif kind == "P":
                            mlp_pre(c)
                        elif kind == "M":
                            mlp_main(c)
                        elif kind == "L":
                            rs_last(c)
                        else:
                            rs_pair(c)
    nc.compile()
    return nc


def _prep(hidden_states, positions, w_in_ln, w_q, w_kv_a, w_kv_a_ln,
          w_kv_b, w_o, w_post_ln, w_gate, w_up, w_down):
    hT = np.ascontiguousarray(
        np.asarray(hidden_states, np.float32).reshape(T, HID).T)

    pos = np.asarray(positions).reshape(-1).astype(np.float64)
    inv = ROPE_BASE ** (-np.arange(0, DR, 2, dtype=np.float64) / DR)
    fr = pos[:, None] * inv[None, :]                      # [T, 32]
    c32 = np.cos(fr).T.astype(np.float32)                 # [32, T]
    s32 = np.sin(fr).T.astype(np.float32)
    cosf = np.concatenate([c32, c32], 0)                  # [64, T]
    sinf = np.concatenate([-s32, s32], 0)

    r = np.arange(P)[:, None]
    c = np.arange(TW)[None, :]
    masks = np.stack([np.where(c >= r + j * P, 0.0, NEG) for j in range(2)],
                     1).astype(np.float32)                # [128, 2, 256]

    w_in_ln = np.asarray(w_in_ln, np.float32)
    wqf = (np.asarray(w_q, np.float32) * w_in_ln[:, None] * SCALING
           ).reshape(HID, H, DQK)
    wkvaf = np.asarray(w_kv_a, np.float32) * w_in_ln[:, None]
    kpe_w = wkvaf[:, KVR:]
    wkva_p = np.concatenate([wkvaf[:, :KVR], kpe_w[:, 0::2], kpe_w[:, 1::2]], 1)
    wkvbf = (np.asarray(w_kv_b, np.float32)
             * np.asarray(w_kv_a_ln, np.float32)[:, None]).reshape(KVR, H, DN + DV)
    w_post_ln = np.asarray(w_post_ln, np.float32)
    wgf = np.asarray(w_gate, np.float32) * w_post_ln[:, None]
    wuf = np.asarray(w_up, np.float32) * w_post_ln[:, None]
    wdf = np.asarray(w_down, np.float32)
    wof = np.asarray(w_o, np.float32).reshape(H, DV, HID)

    hTb = hT.astype(BF)
    wkvbn_full = np.concatenate([wkvbf[:, h, :DN] for h in range(H)], 1).astype(BF)
    wkvbv_full = np.concatenate([wkvbf[:, h, DN:] for h in range(H)], 1).astype(BF)
    in_maps = []
    for core in range(NC_N):
        hs = [2 * core, 2 * core + 1]
        nopes = np.concatenate([wqf[:, h, :DN] for h in hs], 1)
        pes = []
        for h in hs:
            pe = wqf[:, h, DN:]
            pes += [pe[:, 0::2], pe[:, 1::2]]
        wq_c = np.concatenate([nopes] + pes, 1)
        tsl = slice(core * TWC, (core + 1) * TWC)
        in_maps.append({
            "htb": hTb,
            "htc": hTb[:, tsl].copy(),
            "wq": wq_c.astype(BF),
            "wkva": wkva_p.astype(BF),
            "wkvbn": wkvbn_full,
            "wkvbv": wkvbv_full,
            "wo": np.concatenate([wof[h] for h in hs], 0).astype(BF),
            "wg": wgf[:, core * FPC:(core + 1) * FPC].astype(BF),
            "wu": wuf[:, core * FPC:(core + 1) * FPC].astype(BF),
            "wd": wdf[core * FPC:(core + 1) * FPC, :].astype(BF),
            "cosf": cosf.astype(BF),
            "sinf": sinf.astype(BF),
            "csc": cosf[:, tsl].astype(BF),
            "snc": sinf[:, tsl].astype(BF),
            "masks": masks,
        })
    return in_maps


def kernel(**inputs):
    if "nc" not in _CACHE:
        _CACHE["nc"] = _build()
    nc = _CACHE["nc"]
    in_maps = _prep(**inputs)
    res = run_bass_kernel_spmd(nc, in_maps, core_ids=list(range(NC_N)))
    outT = np.concatenate([res.results[c]["mo"] for c in range(NC_N)], 0)
    return np.ascontiguousarray(outT.T).reshape(B, S, HID).astype(np.float32)


# revision 26
# speedup vs baseline: 1.0135x; 1.0135x over previous
"""DeepseekV2 decoder layer on 8 TRN2 NeuronCores (Bass/Tile).

Sharding: TP over heads (2/core) for q/kv_b/attention/o_proj; kv_a + input
norm are token-sharded (each core computes its 256-token slice, then one
AllGather); TP over INTER (1024/core) for the MLP. 8 token chunks of 256;
attention and MLP emission fully interleaved; AllReduce/ReduceScatter run
on chunk PAIRS (512 tokens) to amortize collective overhead while staying
overlapped with compute.

Internal layout is feature-major ("transposed"): activations live as
[feature, token] so every matmul output feeds the next as `rhs` without any
on-device transpose. RoPE pair-swaps, RMSNorm weight folding, the softmax
scaling, and cos/sin tables are all folded into host-side weight prep.
"""

import numpy as np
import ml_dtypes

import concourse.bass as bass
import concourse.mybir as mybir
import concourse.tile as tile
from concourse import bacc
from concourse.bass_utils import run_bass_kernel_spmd

BF = ml_dtypes.bfloat16

B, S, HID = 2, 1024, 2048
T = B * S                      # 2048 tokens
H = 16
DN, DR = 128, 64
DQK = DN + DR
DV = 128
KVR = 512
INTER = 8192
EPS = 1e-6
ROPE_BASE = 10000.0
SCALING = DQK ** -0.5

NC_N = 8
HPC = H // NC_N                # 2 heads per core
FPC = INTER // NC_N            # 1024 inter per core
P = 128
HCH = HID // P                 # 16 hid chunks
HH = HCH // 2                  # 8 hid chunks per ht half
NCH = 8                        # token chunks
TW = T // NCH                  # 256 tokens per chunk
TW2 = 2 * TW                   # 512-token collective pairs
TWC = T // NC_N                # 256 tokens per core (kv_a shard)
QPB = S // TW                  # 4 q-chunks per batch
KVC = KVR // P                 # 4 kv_lora chunks
AGR = KVR + DR + 1             # 577 rows in the kv_a allgather payload
NEG = -30000.0

f32 = mybir.dt.float32
bf16 = mybir.dt.bfloat16
ADD = mybir.AluOpType.add
MUL = mybir.AluOpType.mult
BYP = mybir.AluOpType.bypass
AF = mybir.ActivationFunctionType

_CACHE = {}


def _build():
    nc = bacc.Bacc("TRN2", target_bir_lowering=False, debug=False, num_devices=NC_N)
    dp = lambda n, sh, dt: nc.dram_tensor(n, sh, dt, kind="ExternalInput")
    htb = dp("htb", [HID, T], bf16)
    htc = dp("htc", [HID, TWC], bf16)           # this core's token slice
    wq = dp("wq", [HID, HPC * DQK], bf16)       # [h0n,h1n,h0x1,h0x2,h1x1,h1x2]
    wkva = dp("wkva", [HID, KVR + DR], bf16)
    wkvbn = dp("wkvbn", [KVR, HPC * DN], bf16)
    wkvbv = dp("wkvbv", [KVR, HPC * DV], bf16)
    wo = dp("wo", [HPC * DV, HID], bf16)
    wg = dp("wg", [HID, FPC], bf16)
    wu = dp("wu", [HID, FPC], bf16)
    wd = dp("wd", [FPC, HID], bf16)
    cosf = dp("cosf", [DR, T], bf16)
    sinf = dp("sinf", [DR, T], bf16)
    csc = dp("csc", [DR, TWC], bf16)
    snc = dp("snc", [DR, TWC], bf16)
    masks = dp("masks", [P, 2, TW], f32)
    mo = nc.dram_tensor("mo", [HID // NC_N, T], bf16, kind="ExternalOutput")
    rg = [list(range(NC_N))]

    r2 = lambda ap: ap.rearrange("(o p) t -> p o t", p=P)
    cols = lambda c: slice(c * TW, (c + 1) * TW)

    with tile.TileContext(nc) as tc:
        with tc.tile_pool(name="const", bufs=1) as cpool, \
             tc.tile_pool(name="dram", bufs=1, space="DRAM") as dram, \
             tc.tile_pool(name="wrk", bufs=2) as wrk, \
             tc.tile_pool(name="ps", bufs=1, space="PSUM") as ps:
            ones_col = cpool.tile([P, 1], bf16)
            nc.vector.memset(ones_col[:], 1.0)
            ones_row = cpool.tile([1, P], bf16)
            nc.vector.memset(ones_row[:], 1.0)
            epsb = cpool.tile([1, 1], f32)
            nc.vector.memset(epsb[:], EPS)

            ag_in = dram.tile([KVR, TWC], bf16, name="ag_in")
            ag_out = dram.tile([NC_N * KVR, TWC], bf16, addr_space="Shared",
                               name="ag_out")
            ag2_in = dram.tile([DR + 1, TWC], bf16, name="ag2_in")
            ag2_out = dram.tile([NC_N * (DR + 1), TWC], bf16,
                                addr_space="Shared", name="ag2_out")
            NP = NCH // 2
            ar_in = [dram.tile([HID, TW2], bf16, name=f"ar_in{p}") for p in range(NP)]
            ar_out = [dram.tile([HID, TW2], bf16, addr_space="Shared",
                                name=f"ar_out{p}") for p in range(NP)]
            rs_in = [dram.tile([HID, TW2], bf16, name=f"rs_in{p}") for p in range(NP)]
            rs_out = [dram.tile([HID // NC_N, TW2], bf16, name=f"rs_out{p}")
                      for p in range(NP)]
            rsl_in = [dram.tile([HID, TW], bf16, name=f"rsl_in{j}")
                      for j in range(2)]
            rsl_out = [dram.tile([HID // NC_N, TW], bf16, name=f"rsl_out{j}")
                      for j in range(2)]

            with tc.tile_pool(name="mlpw", bufs=1) as mlpw, \
                 tc.tile_pool(name="keepA", bufs=1) as keepA:
                wg_sb = mlpw.tile([P, HCH, FPC], bf16)       # 32K
                wu_sb = mlpw.tile([P, HCH, FPC], bf16)       # 32K
                wd_sb = mlpw.tile([P, FPC // P, HID], bf16)  # 32K

                qsb = keepA.tile([P, HPC, T], bf16)          # 8K
                qr = [keepA.tile([DR, T], bf16, name=f"qr{h}") for h in range(HPC)]
                krope = keepA.tile([DR, T], bf16)
                knope = keepA.tile([P, HPC, T], bf16)        # 8K
                vnat = keepA.tile([P, T // P, HPC * DV], bf16)  # 8K
                wo_sb = keepA.tile([P, HPC, HID], bf16)      # 8K
                msk = keepA.tile([P, 2, TW], f32)            # 2K
                nc.sync.dma_start(wo_sb[:], r2(wo.ap()))
                nc.sync.dma_start(msk[:], masks.ap())

                # ==== A0: this core's kv_a/r1 shard + AllGather ====
                with tc.tile_pool(name="a1", bufs=1) as a1:
                    cs = a1.tile([DR, T], bf16)
                    sn = a1.tile([DR, T], bf16)
                    qsb2 = a1.tile([P, T], bf16)             # rope-input plane
                    r1row = a1.tile([1, T], bf16)

                    a1l_ctx = tc.tile_pool(name="a1l", bufs=1)
                    a1l = a1l_ctx.__enter__()
                    wkva_sb = a1l.tile([P, HCH, KVR + DR], bf16)  # 18K
                    htl = a1l.tile([P, HCH, TWC], bf16)           # 8K
                    csl = a1l.tile([DR, TWC], bf16)
                    snl = a1l.tile([DR, TWC], bf16)
                    nc.sync.dma_start(htl[:], r2(htc.ap()))
                    nc.sync.dma_start(wkva_sb[:], r2(wkva.ap()))
                    nc.sync.dma_start(csl[:], csc.ap())
                    nc.sync.dma_start(snl[:], snc.ap())
                    nc.sync.dma_start(cs[:], cosf.ap())
                    nc.sync.dma_start(sn[:], sinf.ap())

                    # local input-norm scale r1 for this core's tokens
                    ssl = ps.tile([1, TWC], f32, tag="row", bufs=2, name="ssl")
                    for o in range(HCH):
                        sq = wrk.tile([P, TWC], bf16, tag="sq", bufs=3, name="sql")
                        nc.scalar.square(sq[:], htl[:, o, :])
                        nc.tensor.matmul(ssl[:], ones_col[:], sq[:],
                                         start=(o == 0), stop=(o == HCH - 1))
                    srl = wrk.tile([1, TWC], f32, tag="frow", bufs=3, name="srl")
                    nc.scalar.activation(srl[:], ssl[:], AF.Sqrt,
                                         bias=epsb[:], scale=1.0 / HID)
                    rrl = wrk.tile([1, TWC], f32, tag="frow", bufs=3, name="rrl")
                    nc.vector.reciprocal_approx_fast(out=rrl[:], in_=srl[:])
                    rl = wrk.tile([1, TWC], bf16, tag="rb2", bufs=2, name="rl")
                    nc.vector.tensor_copy(out=rl[:], in_=rrl[:])
                    bclp = ps.tile([P, TWC], f32, tag="gu", bufs=2, name="bclp")
                    nc.tensor.matmul(bclp[:], ones_row[:], rl[:],
                                     start=True, stop=True)
                    bcl = wrk.tile([P, TWC], bf16, tag="bc", bufs=2, name="bcl")
                    nc.vector.tensor_copy(out=bcl[:], in_=bclp[:])

                    # local latent + 2nd rmsnorm + k_pe rope
                    kval = a1l.tile([P, KVC, TWC], bf16, name="kval")
                    kpel = a1l.tile([DR, TWC], bf16, name="kpel")
                    ss2 = ps.tile([1, TWC], f32, tag="row", bufs=2, name="ss2l")
                    for f in range(KVC + 1):
                        wid = P if f < KVC else DR
                        lp = ps.tile([P, TWC], f32, tag="big", bufs=2, name="lpl")
                        for o in range(HCH):
                            nc.tensor.matmul(lp[:wid, :],
                                             wkva_sb[:, o, f * P:f * P + wid],
                                             htl[:, o, :],
                                             start=(o == 0), stop=(o == HCH - 1))
                        if f < KVC:
                            sq2 = wrk.tile([P, TWC], bf16, tag="sq", bufs=3,
                                           name="sq2l")
                            nc.scalar.square(sq2[:], lp[:])
                            nc.tensor.matmul(ss2[:], ones_col[:], sq2[:],
                                             start=(f == 0), stop=(f == KVC - 1))
                            nc.vector.tensor_copy(out=kval[:, f, :], in_=lp[:])
                        else:
                            nc.vector.tensor_tensor(kpel[:], lp[:DR, :],
                                                    bcl[:DR, :], MUL)
                    sr2 = wrk.tile([1, TWC], f32, tag="frow", bufs=3, name="sr2l")
                    nc.scalar.activation(sr2[:], ss2[:], AF.Sqrt,
                                         bias=epsb[:], scale=1.0 / KVR)
                    rr2 = wrk.tile([1, TWC], f32, tag="frow", bufs=3, name="rr2l")
                    nc.vector.reciprocal_approx_fast(out=rr2[:], in_=sr2[:])
                    rb2 = wrk.tile([1, TWC], bf16, tag="rb2", bufs=2, name="rb2l")
                    nc.vector.tensor_copy(out=rb2[:], in_=rr2[:])
                    bcp2 = ps.tile([P, TWC], f32, tag="gu", bufs=2, name="bcp2l")
                    nc.tensor.matmul(bcp2[:], ones_row[:], rb2[:],
                                     start=True, stop=True)
                    bc2 = wrk.tile([P, TWC], bf16, tag="bc", bufs=2, name="bc2l")
                    nc.vector.tensor_copy(out=bc2[:], in_=bcp2[:])
                    for f in range(KVC):
                        nc.vector.tensor_tensor(kval[:, f, :], kval[:, f, :],
                                                bc2[:], MUL)

                    ksl = a1l.tile([DR, TWC], bf16, name="ksl")
                    nc.sync.dma_start(ksl[0:32, :], kpel[32:64, :])
                    nc.sync.dma_start(ksl[32:64, :], kpel[0:32, :])
                    ktl = a1l.tile([DR, TWC], bf16, name="ktl")
                    krl = a1l.tile([DR, TWC], bf16, name="krl")
                    nc.vector.tensor_tensor(ktl[:], kpel[:], csl[:], MUL)
                    nc.vector.tensor_tensor(krl[:], ksl[:], snl[:], MUL)
                    nc.vector.tensor_tensor(krl[:], krl[:], ktl[:], ADD)

                    # pack + allgather: kva first (unblocks kv_b), then
                    # [krope(64); r1(1)]
                    nc.sync.dma_start(
                        ag_in[:, :].rearrange("(f p) t -> p f t", p=P), kval[:])
                    nc.gpsimd.collective_compute(
                        "AllGather", BYP, ins=[ag_in[:].opt()],
                        outs=[ag_out[:].opt()], replica_groups=rg)
                    nc.sync.dma_start(ag2_in[0:DR, :], krl[:])
                    nc.sync.dma_start(ag2_in[DR:DR + 1, :], rl[:])
                    nc.gpsimd.collective_compute(
                        "AllGather", BYP, ins=[ag2_in[:].opt()],
                        outs=[ag2_out[:].opt()], replica_groups=rg)
                    a1l_ctx.__exit__(None, None, None)

                    # ==== A1: q projection over all tokens ====
                    a1p_ctx = tc.tile_pool(name="a1p", bufs=1)
                    a1p = a1p_ctx.__enter__()
                    wq_sb = a1p.tile([P, HCH, HPC * DQK], bf16)   # 12K
                    nc.sync.dma_start(wq_sb[:], r2(wq.ap()))
                    for c in range(NCH):
                        tc0 = cols(c)
                        hts = []
                        for half in range(2):
                            hth = a1p.tile([P, HH, TW], bf16, tag="ht", bufs=3,
                                           name="ht")
                            nc.sync.dma_start(
                                hth[:],
                                r2(htb.ap()[half * P * HH:(half + 1) * P * HH, tc0]))
                            hts.append(hth)
                        ht = lambda o: hts[o // HH][:, o % HH, :]
                        for f in range(3):
                            qp = ps.tile([P, TW], f32, tag="big", bufs=2, name="qp")
                            for o in range(HCH):
                                nc.tensor.matmul(qp[:],
                                                 wq_sb[:, o, f * P:(f + 1) * P],
                                                 ht(o),
                                                 start=(o == 0), stop=(o == HCH - 1))
                            dst = qsb[:, f, tc0] if f < HPC else qsb2[:, tc0]
                            nc.vector.tensor_copy(out=dst, in_=qp[:])

                    # r1/krope rows from the second allgather
                    agv = ag2_out[:, :].rearrange("(c r) t -> r c t", c=NC_N)
                    nc.sync.dma_start(
                        r1row[:].rearrange("p (c t) -> p c t", c=NC_N),
                        agv[DR:DR + 1])
                    nc.sync.dma_start(
                        krope[:].rearrange("p (c t) -> p c t", c=NC_N),
                        agv[0:DR])
                    a1p_ctx.__exit__(None, None, None)

                    # kv_b from the gathered latent (before rope: the rope
                    # DVE work overlaps these matmuls)
                    kb_ctx = tc.tile_pool(name="kb", bufs=1)
                    kb = kb_ctx.__enter__()
                    wkvbn_sb = kb.tile([P, KVC, HPC * DN], bf16)
                    wkvbv_sb = kb.tile([P, KVC, HPC * DV], bf16)
                    nc.sync.dma_start(wkvbn_sb[:], r2(wkvbn.ap()))
                    nc.sync.dma_start(wkvbv_sb[:], r2(wkvbv.ap()))
                    for c in range(NCH):
                        tc0 = cols(c)
                        kvac = kb.tile([P, KVC, TW], bf16, tag="kvac", bufs=2,
                                       name="kvac")
                        nc.sync.dma_start(
                            kvac[:],
                            ag_out[c * KVR:(c + 1) * KVR, :].rearrange(
                                "(f p) t -> p f t", p=P))
                        for h in range(HPC):
                            kp = ps.tile([P, TW], f32, tag="big", bufs=2, name="kp")
                            for f in range(KVC):
                                nc.tensor.matmul(kp[:],
                                                 wkvbn_sb[:, f, h * P:(h + 1) * P],
                                                 kvac[:, f, :],
                                                 start=(f == 0), stop=(f == KVC - 1))
                            nc.vector.tensor_copy(out=knope[:, h, tc0], in_=kp[:])
                        for tsub in range(TW // P):
                            to = c * (TW // P) + tsub
                            vp = ps.tile([P, HPC * DV], f32, tag="att", bufs=1,
                                         name="vp")
                            for f in range(KVC):
                                nc.tensor.matmul(vp[:],
                                                 kvac[:, f, tsub * P:(tsub + 1) * P],
                                                 wkvbv_sb[:, f, :],
                                                 start=(f == 0), stop=(f == KVC - 1))
                            nc.vector.tensor_copy(out=vnat[:, to, :], in_=vp[:])

                    # scale q in place by r1
                    for c in range(NCH):
                        tc0 = cols(c)
                        bcp = ps.tile([P, TW], f32, tag="gu", bufs=2, name="bcq")
                        nc.tensor.matmul(bcp[:], ones_row[:], r1row[:, tc0],
                                         start=True, stop=True)
                        bc1 = wrk.tile([P, TW], bf16, tag="bc", bufs=2, name="bc1")
                        nc.vector.tensor_copy(out=bc1[:], in_=bcp[:])
                        for f in range(HPC):
                            nc.vector.tensor_tensor(qsb[:, f, tc0], qsb[:, f, tc0],
                                                    bc1[:], MUL)
                        nc.vector.tensor_tensor(qsb2[:, tc0], qsb2[:, tc0],
                                                bc1[:], MUL)

                    # q rope: [x1(32); x2(32)] per head; pair-swap via sbuf dma
                    rope_ctx = tc.tile_pool(name="rope", bufs=4)
                    rp = rope_ctx.__enter__()
                    for h in range(HPC):
                        src = qsb2[:, :]
                        if h == 0:
                            direct = src[0:DR, :]
                        else:
                            dcp = rp.tile([DR, T], bf16, tag="rt", name="dcp")
                            nc.sync.dma_start(dcp[:], src[DR:2 * DR, :])
                            direct = dcp[:]
                        sw = rp.tile([DR, T], bf16, tag="rt", name=f"qsw{h}")
                        nc.sync.dma_start(sw[0:32, :], src[h * DR + 32:h * DR + 64, :])
                        nc.sync.dma_start(sw[32:64, :], src[h * DR:h * DR + 32, :])
                        tmp = rp.tile([DR, T], bf16, tag="rt", name=f"qtmp{h}")
                        nc.vector.tensor_tensor(tmp[:], direct, cs[:], MUL)
                        nc.vector.tensor_tensor(qr[h][:], sw[:], sn[:], MUL)
                        nc.vector.tensor_tensor(qr[h][:], qr[h][:], tmp[:], ADD)
                    rope_ctx.__exit__(None, None, None)
                    kb_ctx.__exit__(None, None, None)

                # ============ fused attention + MLP ============
                with tc.tile_pool(name="aliv", bufs=1) as aliv, \
                     tc.tile_pool(name="mliv", bufs=1) as mliv:

                    def attn_chunk(c):
                        b, qt = c // QPB, c % QPB
                        qc0 = cols(c)
                        nkt = 2 * qt + 2
                        attn = aliv.tile([P, HPC, TW], bf16, tag="at", bufs=2,
                                         name="attn")
                        for h in range(HPC):
                            dnp = ps.tile([1, TW], f32, tag="row", bufs=2,
                                          name="dnp")
                            atp = ps.tile([P, TW], f32, tag="att", bufs=1,
                                          name="atp")
                            exs = [None] * nkt

                            def consume(kt):
                                nc.tensor.matmul(dnp[:], ones_col[:], exs[kt][:],
                                                 start=(kt == 0),
                                                 stop=(kt == nkt - 1))
                                nc.tensor.matmul(atp[:],
                                                 vnat[:, b * (S // P) + kt,
                                                      h * DV:(h + 1) * DV],
                                                 exs[kt][:],
                                                 start=(kt == 0),
                                                 stop=(kt == nkt - 1))

                            for kt in range(nkt):
                                kc0 = b * S + kt * P
                                scp = ps.tile([P, TW], f32, tag="big", bufs=2,
                                              name="scp")
                                nc.tensor.matmul(scp[:],
                                                 knope[:, h, kc0:kc0 + P],
                                                 qsb[:, h, qc0],
                                                 start=True, stop=False)
                                nc.tensor.matmul(scp[:],
                                                 krope[:, kc0:kc0 + P],
                                                 qr[h][:, qc0],
                                                 start=False, stop=True)
                                ex = wrk.tile([P, TW], bf16, tag="ex", bufs=4,
                                              name="ex")
                                j = kt - 2 * qt
                                if j >= 0:
                                    mtmp = wrk.tile([P, TW], f32, tag="mt",
                                                    bufs=2, name="mtmp")
                                    nc.vector.tensor_tensor(mtmp[:], scp[:],
                                                            msk[:, j, :], ADD)
                                    nc.scalar.activation(ex[:], mtmp[:], AF.Exp)
                                else:
                                    nc.scalar.activation(ex[:], scp[:], AF.Exp)
                                exs[kt] = ex
                                if kt >= 2:
                                    consume(kt - 2)
                            consume(max(nkt - 2, 0))
                            if nkt > 1:
                                consume(nkt - 1)
                            drow = wrk.tile([1, TW], f32, tag="frow", bufs=3,
                                            name="drow")
                            nc.vector.reciprocal_approx_fast(out=drow[:],
                                                             in_=dnp[:])
                            rbd = wrk.tile([1, TW], bf16, tag="rb2", bufs=2,
                                           name="rbd")
                            nc.vector.tensor_copy(out=rbd[:], in_=drow[:])
                            dbp = ps.tile([P, TW], f32, tag="gu", bufs=2,
                                          name="dbp")
                            nc.tensor.matmul(dbp[:], ones_row[:], rbd[:],
                                             start=True, stop=True)
                            dbc = wrk.tile([P, TW], f32, tag="mt", bufs=2,
                                           name="dbc")
                            nc.vector.tensor_copy(out=dbc[:], in_=dbp[:])
                            nc.vector.tensor_tensor(attn[:, h, :], atp[:],
                                                    dbc[:], MUL)
                        # o_proj partials into this pair's AR staging half
                        stg = mliv.tile([P, HCH, TW], bf16, tag="stg", bufs=1,
                                        name="ostg")
                        for ho in range(HCH):
                            op = ps.tile([P, TW], f32, tag="big", bufs=2,
                                         name="op")
                            for h in range(HPC):
                                nc.tensor.matmul(op[:],
                                                 wo_sb[:, h, ho * P:(ho + 1) * P],
                                                 attn[:, h, :],
                                                 start=(h == 0),
                                                 stop=(h == HPC - 1))
                            nc.vector.tensor_copy(out=stg[:, ho, :], in_=op[:])
                        half = (c % 2) * TW
                        nc.sync.dma_start(
                            r2(ar_in[c // 2][:, half:half + TW]), stg[:])

                    def ar_pair(p):
                        nc.gpsimd.collective_compute(
                            "AllReduce", ADD, ins=[ar_in[p][:].opt()],
                            outs=[ar_out[p][:].opt()], replica_groups=rg)

                    h2s, xs = {}, {}

                    def mlp_pre(c):
                        tc0 = cols(c)
                        half = (c % 2) * TW
                        # x = hidden + attn_out (in place over the AR dma)
                        x = mliv.tile([P, HCH, TW], bf16, tag="x", bufs=2,
                                      name="x")
                        nc.sync.dma_start(x[:],
                                          r2(ar_out[c // 2][:, half:half + TW]))
                        htre = mliv.tile([P, HCH, TW], bf16, tag="htre", bufs=1,
                                         name="htre")
                        nc.sync.dma_start(htre[:], r2(htb.ap()[:, tc0]))
                        nc.vector.tensor_tensor(x[:, 0:HH, :], x[:, 0:HH, :],
                                                htre[:, 0:HH, :], ADD)
                        nc.vector.tensor_tensor(x[:, HH:HCH, :], x[:, HH:HCH, :],
                                                htre[:, HH:HCH, :], ADD)
                        # post-attn rmsnorm scale
                        ssp3 = ps.tile([1, TW], f32, tag="row", bufs=2,
                                       name="ssp3")
                        for o in range(HCH):
                            sq3 = wrk.tile([P, TW], bf16, tag="sq", bufs=3,
                                           name="sq3")
                            nc.scalar.square(sq3[:], x[:, o, :])
                            nc.tensor.matmul(ssp3[:], ones_col[:], sq3[:],
                                             start=(o == 0), stop=(o == HCH - 1))
                        srow3 = wrk.tile([1, TW], f32, tag="frow", bufs=3,
                                         name="srow3")
                        nc.scalar.activation(srow3[:], ssp3[:], AF.Sqrt,
                                             bias=epsb[:], scale=1.0 / HID)
                        rrow3 = wrk.tile([1, TW], f32, tag="frow", bufs=3,
                                         name="rrow3")
                        nc.vector.reciprocal_approx_fast(out=rrow3[:],
                                                         in_=srow3[:])
                        rb3 = wrk.tile([1, TW], bf16, tag="rb2", bufs=2,
                                       name="rb3")
                        nc.vector.tensor_copy(out=rb3[:], in_=rrow3[:])
                        bcp3 = ps.tile([P, TW], f32, tag="gu", bufs=2,
                                       name="bcp3")
                        nc.tensor.matmul(bcp3[:], ones_row[:], rb3[:],
                                         start=True, stop=True)
                        bc3 = wrk.tile([P, TW], bf16, tag="bc", bufs=2,
                                       name="bc3")
                        nc.vector.tensor_copy(out=bc3[:], in_=bcp3[:])
                        h2 = mliv.tile([P, HCH, TW], bf16, tag="h2", bufs=2,
                                       name="h2")
                        for o in range(HCH):
                            nc.vector.tensor_tensor(h2[:, o, :], x[:, o, :],
                                                    bc3[:], MUL)
                        # x -> x/8 in place (folded residual for ReduceScatter)
                        nc.vector.tensor_scalar_mul(x[:], x[:], 0.125)
                        h2s[c], xs[c] = h2, x

                    def mlp_main(c):
                        h2, x = h2s.pop(c), xs.pop(c)
                        half = (c % 2) * TW
                        act = mliv.tile([P, FPC // P, TW], bf16, tag="act",
                                        bufs=1, name="act")
                        for fi in range(FPC // P):
                            gp = ps.tile([P, TW], f32, tag="gu", bufs=2,
                                         name="gp")
                            for o in range(HCH):
                                nc.tensor.matmul(gp[:],
                                                 wg_sb[:, o, fi * P:(fi + 1) * P],
                                                 h2[:, o, :],
                                                 start=(o == 0),
                                                 stop=(o == HCH - 1))
                            up = ps.tile([P, TW], f32, tag="gu", bufs=2,
                                         name="up")
                            for o in range(HCH):
                                nc.tensor.matmul(up[:],
                                                 wu_sb[:, o, fi * P:(fi + 1) * P],
                                                 h2[:, o, :],
                                                 start=(o == 0),
                                                 stop=(o == HCH - 1))
                            gs = wrk.tile([P, TW], bf16, tag="gs", bufs=2,
                                          name="gs")
                            nc.scalar.activation(gs[:], gp[:], AF.Silu)
                            nc.vector.tensor_tensor(act[:, fi, :], up[:],
                                                    gs[:], MUL)

                        # down projection partial (+x/8) into RS staging half
                        stg = mliv.tile([P, HCH, TW], bf16, tag="stg", bufs=1,
                                        name="dstg")
                        for ho in range(HCH):
                            dpp = ps.tile([P, TW], f32, tag="big", bufs=2,
                                          name="dpp")
                            for f in range(FPC // P):
                                nc.tensor.matmul(dpp[:],
                                                 wd_sb[:, f, ho * P:(ho + 1) * P],
                                                 act[:, f, :],
                                                 start=(f == 0),
                                                 stop=(f == FPC // P - 1))
                            nc.vector.tensor_tensor(stg[:, ho, :], dpp[:],
                                                    x[:, ho, :], ADD)
                        if c >= 6:
                            nc.sync.dma_start(r2(rsl_in[c - 6][:, :]), stg[:])
                        else:
                            nc.sync.dma_start(
                                r2(rs_in[c // 2][:, half:half + TW]), stg[:])

                    def rs_pair(p):
                        nc.gpsimd.collective_compute(
                            "ReduceScatter", ADD, ins=[rs_in[p][:].opt()],
                            outs=[rs_out[p][:].opt()], replica_groups=rg)
                        nc.gpsimd.dma_start(
                            mo.ap()[:, p * TW2:(p + 1) * TW2], rs_out[p][:, :])

                    def rs_last(j):
                        c = 6 + j
                        nc.gpsimd.collective_compute(
                            "ReduceScatter", ADD,
                            ins=[rsl_in[j][:].opt()],
                            outs=[rsl_out[j][:].opt()], replica_groups=rg)
                        nc.gpsimd.dma_start(
                            mo.ap()[:, cols(c)], rsl_out[j][:, :])

                    nc.sync.dma_start(wg_sb[:], r2(wg.ap()))
                    nc.sync.dma_start(wu_sb[:], r2(wu.ap()))
                    nc.sync.dma_start(wd_sb[:], r2(wd.ap()))

                    sched = [("A", 0), ("A", 1), ("R", 0), ("A", 2), ("A", 3),
                             ("R", 1), ("P", 0), ("M", 0), ("A", 4), ("P", 1),
                             ("M", 1), ("A", 5), ("R", 2), ("P", 2),
                             ("M", 2), ("A", 6), ("P", 3), ("M", 3),
                             ("A", 7), ("R", 3), ("P", 4), ("M", 4), ("P", 5),
                             ("M", 5), ("P", 6), ("M", 6), ("S", 0), ("S", 1),
                             ("S", 2), ("L", 0), ("P", 7), ("M", 7), ("L", 1)]
                    for kind, c in sched:
                        if kind == "A":
                            attn_chunk(c)
                        elif kind == "R":
                            ar_pair(c)
                        elif kind == "P":
                            mlp_pre(c)
                        elif kind == "M":
                            mlp_main(c)
                        elif kind == "L":
                            rs_last(c)
                        else:
                            rs_pair(c)
    nc.compile()
    return nc


def _prep(hidden_states, positions, w_in_ln, w_q, w_kv_a, w_kv_a_ln,
          w_kv_b, w_o, w_post_ln, w_gate, w_up, w_down):
    hT = np.ascontiguousarray(
        np.asarray(hidden_states, np.float32).reshape(T, HID).T)

    pos = np.asarray(positions).reshape(-1).astype(np.float64)
    inv = ROPE_BASE ** (-np.arange(0, DR, 2, dtype=np.float64) / DR)
    fr = pos[:, None] * inv[None, :]                      # [T, 32]
    c32 = np.cos(fr).T.astype(np.float32)                 # [32, T]
    s32 = np.sin(fr).T.astype(np.float32)
    cosf = np.concatenate([c32, c32], 0)                  # [64, T]
    sinf = np.concatenate([-s32, s32], 0)

    r = np.arange(P)[:, None]
    c = np.arange(TW)[None, :]
    masks = np.stack([np.where(c >= r + j * P, 0.0, NEG) for j in range(2)],
                     1).astype(np.float32)                # [128, 2, 256]

    w_in_ln = np.asarray(w_in_ln, np.float32)
    wqf = (np.asarray(w_q, np.float32) * w_in_ln[:, None] * SCALING
           ).reshape(HID, H, DQK)
    wkvaf = np.asarray(w_kv_a, np.float32) * w_in_ln[:, None]
    kpe_w = wkvaf[:, KVR:]
    wkva_p = np.concatenate([wkvaf[:, :KVR], kpe_w[:, 0::2], kpe_w[:, 1::2]], 1)
    wkvbf = (np.asarray(w_kv_b, np.float32)
             * np.asarray(w_kv_a_ln, np.float32)[:, None]).reshape(KVR, H, DN + DV)
    w_post_ln = np.asarray(w_post_ln, np.float32)
    wgf = np.asarray(w_gate, np.float32) * w_post_ln[:, None]
    wuf = np.asarray(w_up, np.float32) * w_post_ln[:, None]
    wdf = np.asarray(w_down, np.float32)
    wof = np.asarray(w_o, np.float32).reshape(H, DV, HID)

    hTb = hT.astype(BF)
    in_maps = []
    for core in range(NC_N):
        hs = [2 * core, 2 * core + 1]
        nopes = np.concatenate([wqf[:, h, :DN] for h in hs], 1)
        pes = []
        for h in hs:
            pe = wqf[:, h, DN:]
            pes += [pe[:, 0::2], pe[:, 1::2]]
        wq_c = np.concatenate([nopes] + pes, 1)
        tsl = slice(core * TWC, (core + 1) * TWC)
        in_maps.append({
            "htb": hTb,
            "htc": hTb[:, tsl].copy(),
            "wq": wq_c.astype(BF),
            "wkva": wkva_p.astype(BF),
            "wkvbn": np.concatenate([wkvbf[:, h, :DN] for h in hs], 1).astype(BF),
            "wkvbv": np.concatenate([wkvbf[:, h, DN:] for h in hs], 1).astype(BF),
            "wo": np.concatenate([wof[h] for h in hs], 0).astype(BF),
            "wg": wgf[:, core * FPC:(core + 1) * FPC].astype(BF),
            "wu": wuf[:, core * FPC:(core + 1) * FPC].astype(BF),
            "wd": wdf[core * FPC:(core + 1) * FPC, :].astype(BF),
            "cosf": cosf.astype(BF),
            "sinf": sinf.astype(BF),
            "csc": cosf[:, tsl].astype(BF),
            "snc": sinf[:, tsl].astype(BF),
            "masks": masks,
        })
    return in_maps


def kernel(**inputs):
    if "nc" not in _CACHE:
        _CACHE["nc"] = _build()
    nc = _CACHE["nc"]
    in_maps = _prep(**inputs)
    res = run_bass_kernel_spmd(nc, in_maps, core_ids=list(range(NC_N)))
    outT = np.concatenate([res.results[c]["mo"] for c in range(NC_N)], 0)
    return np.ascontiguousarray(outT.T).reshape(B, S, HID).astype(np.float32)
